# revision 15
# baseline (speedup 1.0000x reference)
"""Attentional pooling mechanism on 8 TRN2 NeuronCores.

Math (per batch sample b):
    AP  = tanh(A @ W.T + b)                      [256]
    R_t = B[b, t, :] . AP                        [8192]
    e_t = exp(-R_t)
    out = (sum_t e_t * B[b, t, :]) / (sum_t e_t) [256]

Sharding: pure data parallel, batch 64 -> 8 cores x 8 samples, weights
replicated. Single fused pass over B (the reference uses unnormalized
exp(-R), so numerator and denominator accumulate in one sweep).

Per-core pipeline (per sample: 64 t-tiles of [128 t, 256 h], groups of 8):
    DMA 1 MiB group (SWDGE, 8 buffers in flight), then scores split across
    engines to balance them:
      - tiles 2..7: DVE scalar_tensor_tensor fused dot with accum_out:
        R[t] = sum_h B[t,h]*AP[h] against a partition-broadcast AP (fp32)
      - tiles 0..1: PE path: fp32 transposes -> ACT copy -> fp32 matmul
        (AP column stationary) -> row exp -> tiny e transposes
    -> ACT exp(-R) -> float32r e directly, accum_out accumulates den partials
    -> PE float32r matmuls (lhsT = e column): num[h] += sum_t e_t B[t,h],
       accumulated in PSUM across the whole sample
    -> sample tail (deferred into the next sample's stream): den reduce,
       reciprocal, scale -> single output DMA at the end.

Measured on trn2 (slope of in-kernel repeats): ~196 us per pass; the
DMA-only floor measures ~131 us, fp32 DVE dot stream ~155 us.

Wait discipline: this codegen allows at most ONE sync wait per instruction.
Absorber ops consume semaphores onto the right engines, PSUM banks have a
single reader engine, and buffer counts are sized so write-after-read
dependencies are already covered by each engine's observed clock.
"""
import numpy as np
from contextlib import ExitStack

import concourse.bacc as bacc
import concourse.tile as tile
import concourse.mybir as mybir
from concourse.bass_utils import run_bass_kernel_spmd
from concourse.masks import make_identity

F32 = mybir.dt.float32
F32R = mybir.dt.float32r
EXP = mybir.ActivationFunctionType.Exp
TANH = mybir.ActivationFunctionType.Tanh
MUL = mybir.AluOpType.mult
ADD = mybir.AluOpType.add

BATCH, T, FEAT, HID = 64, 8192, 512, 256
NCORES = 8
SPC = BATCH // NCORES          # samples per core
NT = T // 128                  # 64 t-tiles per sample
GRP = 8                        # t-tiles per group
NG = NT // GRP                 # 8 groups per sample


def build_nc(reps: int = 1, dma_only: bool = False):
    nc = bacc.Bacc("TRN2", target_bir_lowering=False, debug=False)

    A = nc.dram_tensor("A", [SPC, FEAT], F32, kind="ExternalInput").ap()
    B = nc.dram_tensor("B", [SPC, T, HID], F32R, kind="ExternalInput").ap()
    W = nc.dram_tensor("W", [HID, FEAT], F32, kind="ExternalInput").ap()
    BIAS = nc.dram_tensor("BIAS", [HID], F32, kind="ExternalInput").ap()
    OUT = nc.dram_tensor("OUT", [SPC, HID], F32, kind="ExternalOutput").ap()

    Bv = B.rearrange("s (g j p) h -> s g p j h", g=NG, j=GRP, p=128)
    Wv = W.rearrange("(c p) f -> c p f", p=128)
    BIASv = BIAS.rearrange("(a h) -> a h", a=1)

    with tile.TileContext(nc) as tc, ExitStack() as ctx:
        cst = ctx.enter_context(tc.tile_pool(name="cst", bufs=1))
        ps_scr = ctx.enter_context(tc.tile_pool(name="ps_scr", bufs=1, space="PSUM"))

        ident = cst.tile([128, 128], F32, tag="ident")
        make_identity(nc, ident[:])
        zbias = cst.tile([128, 1], F32, tag="zbias")
        nc.gpsimd.memset(zbias[:], 0.0)
        ones_c = cst.tile([128, 1], F32, tag="ones_c")
        nc.gpsimd.memset(ones_c[:], 1.0)
        ones_row = cst.tile([1, 128], F32, tag="ones_row")
        nc.gpsimd.memset(ones_row[:], 1.0)
        act_scr = cst.tile([1, 1], F32, tag="act_scr")
        pool_scr = cst.tile([1, 1], F32, tag="pool_scr")
        dve_mark = cst.tile([1, 1], F32, tag="dve_mark")
        nc.gpsimd.memset(dve_mark[:], 0.0)
        # sel_all[k, s*128 + m] = 1 if k == s else 0 (per-sample broadcast selector)
        sel_all = cst.tile([SPC, SPC * 128], F32, tag="sel_all")
        nc.gpsimd.memset(sel_all[:], 0.0)
        nc.gpsimd.affine_select(
            out=sel_all[:].rearrange("k (s m) -> k s m", s=SPC),
            in_=sel_all[:].rearrange("k (s m) -> k s m", s=SPC),
            compare_op=mybir.AluOpType.not_equal,
            fill=1.0,
            base=0,
            pattern=[[-1, SPC], [0, 128]],
            channel_multiplier=1,
        )

        w_sb = cst.tile([128, 2 * FEAT], F32, tag="w_sb")
        nc.gpsimd.dma_start(w_sb[:, 0:FEAT], Wv[0])
        nc.gpsimd.dma_start(w_sb[:, FEAT : 2 * FEAT], Wv[1])
        a_sb = cst.tile([SPC, FEAT], F32, tag="a_sb")
        nc.gpsimd.dma_start(a_sb[:], A)
        b_row = cst.tile([1, HID], F32, tag="b_row")
        nc.gpsimd.dma_start(b_row[:], BIASv)

        scr = ps_scr.tile([1, 1], F32, tag="scr")

        def absorb(ap):
            nc.tensor.transpose(scr[:], ap, ident[0:1, 0:1])

        # ACT absorber for gpsimd-made constants (also loads the exp table set)
        nc.scalar.activation(act_scr[:], zbias[0:1, 0:1], EXP, bias=zbias[0:1, 0:1])
        # PE absorbers: gpsimd consts + the input DMA queues
        absorb(ident[0:1, 0:1])
        absorb(w_sb[0:1, 0:1])
        absorb(w_sb[0:1, FEAT : FEAT + 1])
        absorb(a_sb[0:1, 0:1])
        absorb(b_row[0:1, 0:1])

        # ---- AP = tanh(A @ W.T + bias) ----
        with tc.tile_pool(name="setup_ps", bufs=1, space="PSUM") as sps, \
             tc.tile_pool(name="setup_sb", bufs=1) as ssb:
            wt_sb = ssb.tile([128, 4 * HID], F32, tag="wt_sb")
            for fc in range(4):
                wt_ps = sps.tile([128, HID], F32, tag="wt_ps", name=f"wt_ps{fc}", bufs=2)
                for c in range(2):
                    nc.tensor.transpose(
                        wt_ps[:, c * 128 : (c + 1) * 128],
                        w_sb[:, c * FEAT + fc * 128 : c * FEAT + (fc + 1) * 128],
                        ident[:],
                    )
                nc.vector.tensor_copy(wt_sb[:, fc * HID : (fc + 1) * HID], wt_ps[:])

            at_ps = sps.tile([128, 4 * SPC], F32, tag="at_ps")
            for fc in range(4):
                nc.tensor.transpose(
                    at_ps[:, fc * SPC : (fc + 1) * SPC],
                    a_sb[0:SPC, fc * 128 : (fc + 1) * 128],
                    ident[0:SPC, 0:SPC],
                )
            at_sb = ssb.tile([128, 4 * SPC], F32, tag="at_sb")
            nc.vector.tensor_copy(at_sb[:], at_ps[:])

            ap_ps = sps.tile([SPC, HID], F32, tag="ap_ps")
            for fc in range(4):
                nc.tensor.matmul(
                    ap_ps[:],
                    at_sb[:, fc * SPC : (fc + 1) * SPC],
                    wt_sb[:, fc * HID : (fc + 1) * HID],
                    start=(fc == 0),
                    stop=False,
                )
            nc.tensor.matmul(ap_ps[:], ones_row[:, 0:SPC], b_row[:], start=False, stop=True)

            ap_sb = cst.tile([SPC, HID], F32, tag="ap_sb")
            nc.scalar.activation(ap_sb[:], ap_ps[:], TANH, bias=zbias[0:SPC])

            # broadcast every AP row to 128 partitions now: sel_s.T @ ap_sb
            apb_all = cst.tile([128, SPC * HID], F32, tag="apb_all")
            for _s in range(SPC):
                apb_ps = sps.tile([128, HID], F32, tag="apb_ps", name=f"apb_ps{_s}", bufs=2)
                nc.tensor.matmul(
                    apb_ps[:],
                    sel_all[0:SPC, _s * 128 : (_s + 1) * 128],
                    ap_sb[:],
                )
                nc.vector.tensor_copy(apb_all[:, _s * HID : (_s + 1) * HID], apb_ps[:])

            # AP transposed to [h, s] for the PE score path
            apt_ps = sps.tile([128, 2 * SPC], F32, tag="apt_ps")
            for hc in range(2):
                nc.tensor.transpose(
                    apt_ps[:, hc * SPC : (hc + 1) * SPC],
                    ap_sb[0:SPC, hc * 128 : (hc + 1) * 128],
                    ident[0:SPC, 0:SPC],
                )
            apt_sb = cst.tile([128, 2 * SPC], F32, tag="apt_sb")
            nc.vector.tensor_copy(apt_sb[:], apt_ps[:])

        # ---- main pools ----
        bpool = ctx.enter_context(tc.tile_pool(name="bpool", bufs=8))
        rpool = ctx.enter_context(tc.tile_pool(name="rpool", bufs=2))
        erpool = ctx.enter_context(tc.tile_pool(name="erpool", bufs=NG))
        small = ctx.enter_context(tc.tile_pool(name="small", bufs=2))
        con_pool = ctx.enter_context(tc.tile_pool(name="con", bufs=1))
        junk_pool = ctx.enter_context(tc.tile_pool(name="junk", bufs=1))
        g_pool = ctx.enter_context(tc.tile_pool(name="g_ps", bufs=2, space="PSUM"))
        tp_pool = ctx.enter_context(tc.tile_pool(name="tp_ps", bufs=2, space="PSUM"))
        rrow_pool = ctx.enter_context(tc.tile_pool(name="rrow_ps", bufs=2, space="PSUM"))
        eps_pool = ctx.enter_context(tc.tile_pool(name="e_ps", bufs=1, space="PSUM"))
        btg_pool = ctx.enter_context(tc.tile_pool(name="btg", bufs=2))
        erow_pool = ctx.enter_context(tc.tile_pool(name="erow", bufs=2))

        con_wide = con_pool.tile([1, SPC * HID], F32, tag="con_wide")
        junk = junk_pool.tile([128, HID], F32, tag="junk")

        started = False
        pending_tail = None
        for rep in range(reps):
            for s in range(SPC):
                if started and not dma_only:
                    prev = (s + SPC - 1) % SPC
                    absorb(con_wide[0:1, prev * HID : prev * HID + 1])

                ap_bcast = apb_all[:, s * HID : (s + 1) * HID]

                dacc = small.tile([128, NG], F32, tag="dacc")
                dacc2 = small.tile([1, NG], F32, tag="dacc2")
                G = g_pool.tile([1, HID + 8], F32, tag="G")

                prev_grp = None  # (e_r, bgrp, g) of previous group
                for g in range(NG):
                    if g == 1 and pending_tail is not None:
                        pending_tail()
                        pending_tail = None
                    if (started or g > 0) and not dma_only:
                        # absorb DVE tick on Pool so the B load carries only
                        # the PE (pooling WAR) wait
                        nc.gpsimd.tensor_copy(pool_scr[:], junk[0:1, 0:1])
                    bgrp = bpool.tile([128, GRP * HID], F32R, tag="bgrp")
                    nc.gpsimd.dma_start(
                        bgrp[:].rearrange("p (j h) -> p j h", j=GRP), Bv[s, g]
                    )

                    if dma_only:
                        prev_grp = None
                        continue
                    e_r = erpool.tile([128, GRP], F32R, tag="e_r")

                    # --- PE score path for tiles j=0,1 ---
                    # transposes laid out c-major so each h-chunk is contiguous
                    tp_ps = tp_pool.tile([128, 512], F32, tag="tp_ps")
                    for c in range(2):
                        for j in range(2):
                            nc.tensor.transpose(
                                tp_ps[:, (c * 2 + j) * 128 : (c * 2 + j + 1) * 128],
                                bgrp[:, j * HID + c * 128 : j * HID + (c + 1) * 128].bitcast(F32),
                                ident[:],
                            )
                    btg = btg_pool.tile([128, 512], F32, tag="btg")
                    nc.scalar.copy(btg[:], tp_ps[:])
                    r_row = rrow_pool.tile([1, 256], F32, tag="r_row")
                    for c in range(2):
                        nc.tensor.matmul(
                            r_row[:],
                            apt_sb[:, c * SPC + s : c * SPC + s + 1],
                            btg[:, c * 256 : (c + 1) * 256],
                            start=(c == 0),
                            stop=(c == 1),
                            skip_group_check=True,
                        )
                    e_row = erow_pool.tile([1, 256], F32, tag="e_row")
                    nc.scalar.activation(
                        e_row[:], r_row[:], EXP, scale=-1.0, bias=zbias[0:1],
                        accum_out=dacc2[0:1, g : g + 1],
                    )
                    e_ps = eps_pool.tile([128, 2], F32, tag="e_ps")
                    for j in range(2):
                        nc.tensor.transpose(
                            e_ps[:, j : j + 1],
                            e_row[0:1, j * 128 : (j + 1) * 128],
                            ident[0:1, 0:1],
                        )
                    for j in range(2):
                        nc.scalar.copy(e_r[:, j : j + 1], e_ps[:, j : j + 1])

                    # --- DVE score path for tiles j=2..7 ---
                    r_sb = rpool.tile([128, GRP], F32, tag="r_sb")
                    for j in range(2, GRP):
                        if j == 4 and prev_grp is not None:
                            per_, pbgrp, pg = prev_grp
                            for jj in range(GRP):
                                nc.tensor.matmul(
                                    G[0:1, 0:HID],
                                    per_[:, jj : jj + 1],
                                    pbgrp[:, jj * HID : (jj + 1) * HID],
                                    start=(pg == 0 and jj == 0),
                                    stop=False,
                                    skip_group_check=True,
                                )
                        nc.vector.scalar_tensor_tensor(
                            out=junk[:],
                            in0=bgrp[:, j * HID : (j + 1) * HID].bitcast(F32),
                            scalar=1.0,
                            in1=ap_bcast[:],
                            op0=MUL,
                            op1=MUL,
                            accum_out=r_sb[:, j : j + 1],
                        )

                    nc.scalar.activation(
                        e_r[:, 2:GRP], r_sb[:, 2:GRP], EXP, scale=-1.0, bias=zbias[:],
                        accum_out=dacc[:, g : g + 1],
                    )
                    prev_grp = (e_r, bgrp, g)
                    started = True

                # flush last group's pooling
                if dma_only:
                    continue
                per_, pbgrp, pg = prev_grp
                for jj in range(GRP):
                    nc.tensor.matmul(
                        G[0:1, 0:HID],
                        per_[:, jj : jj + 1],
                        pbgrp[:, jj * HID : (jj + 1) * HID],
                        start=False,
                        stop=False,
                        skip_group_check=True,
                    )

                # den/scale tail: defer into the next sample's stream
                def make_tail(dacc=dacc, dacc2=dacc2, G=G, s=s):
                    def tail():
                        dsum = small.tile([128, 1], F32, tag="dsum", name="dsum")
                        nc.vector.tensor_reduce(
                            dsum[:], dacc[:], axis=mybir.AxisListType.X, op=ADD
                        )
                        nc.tensor.matmul(
                            G[0:1, HID : HID + 1], ones_c[:], dsum[:],
                            start=False, stop=False, skip_group_check=True,
                        )
                        d2s = small.tile([1, 1], F32, tag="d2s", name="d2s")
                        nc.vector.tensor_reduce(
                            d2s[:], dacc2[:], axis=mybir.AxisListType.X, op=ADD
                        )
                        nc.tensor.matmul(
                            G[0:1, HID : HID + 1], ones_c[0:1, :], d2s[:],
                            start=False, stop=True, skip_group_check=True,
                        )
                        inv = small.tile([1, 1], F32, tag="inv", name="inv")
                        nc.vector.reciprocal(inv[:], G[0:1, HID : HID + 1])
                        nc.vector.tensor_scalar_mul(
                            con_wide[0:1, s * HID : (s + 1) * HID], G[0:1, 0:HID], inv[:]
                        )
                        nc.vector.tensor_copy(dve_mark[:], inv[:])
                    return tail
                pending_tail = make_tail()

        if pending_tail is not None:
            pending_tail()
            pending_tail = None
        if dma_only:
            nc.vector.tensor_copy(con_wide[0:1, 0:1], zbias[0:1, 0:1])
        OUTv = OUT.rearrange("s h -> (s h)").rearrange("(a n) -> a n", a=1)
        nc.gpsimd.dma_start(OUTv, con_wide[0:1, :])

    nc.compile()
    return nc


_NC_CACHE = {}


def _get_nc(reps: int = 1):
    if reps not in _NC_CACHE:
        _NC_CACHE[reps] = build_nc(reps)
    return _NC_CACHE[reps]


def kernel(A, B, W, b):
    A = np.asarray(A, dtype=np.float32)
    B = np.asarray(B, dtype=np.float32)
    W = np.asarray(W, dtype=np.float32)
    b = np.asarray(b, dtype=np.float32)

    nc = _get_nc(1)
    in_maps = [
        {
            "A": A[c * SPC : (c + 1) * SPC],
            "B": B[c * SPC : (c + 1) * SPC],
            "W": W,
            "BIAS": b,
        }
        for c in range(NCORES)
    ]
    res = run_bass_kernel_spmd(nc, in_maps, list(range(NCORES))).results
    out = np.concatenate([r["OUT"] for r in res], axis=0)
    return out[:, None, :].astype(np.float32)


# revision 17
# speedup vs baseline: 24.0277x; 24.0277x over previous
"""Attentional pooling mechanism on 8 TRN2 NeuronCores.

Math (per batch sample b):
    AP  = tanh(A @ W.T + b)                      [256]
    R_t = B[b, t, :] . AP                        [8192]
    e_t = exp(-R_t)
    out = (sum_t e_t * B[b, t, :]) / (sum_t e_t) [256]

Sharding: pure data parallel, batch 64 -> 8 cores x 8 samples, weights
replicated. Single fused pass over B (the reference uses unnormalized
exp(-R), so numerator and denominator accumulate in one sweep).

Per-core pipeline (per sample: 64 t-tiles of [128 t, 256 h], groups of 8):
    DMA 1 MiB group (SWDGE, 8 buffers in flight), then scores split across
    engines to balance them:
      - tiles 2..7: DVE scalar_tensor_tensor fused dot with accum_out:
        R[t] = sum_h B[t,h]*AP[h] against a partition-broadcast AP (fp32)
      - tiles 0..1: PE path: fp32 transposes -> ACT copy -> fp32 matmul
        (AP column stationary) -> row exp -> tiny e transposes
    -> ACT exp(-R) -> float32r e directly, accum_out accumulates den partials
    -> PE float32r matmuls (lhsT = e column): num[h] += sum_t e_t B[t,h],
       accumulated in PSUM across the whole sample
    -> sample tail (deferred into the next sample's stream): den reduce,
       reciprocal, scale -> single output DMA at the end.

Measured on trn2 (slope of in-kernel repeats): ~196 us per pass; the
DMA-only floor measures ~131 us, fp32 DVE dot stream ~155 us.

Wait discipline: this codegen allows at most ONE sync wait per instruction.
Absorber ops consume semaphores onto the right engines, PSUM banks have a
single reader engine, and buffer counts are sized so write-after-read
dependencies are already covered by each engine's observed clock.
"""
import numpy as np
from contextlib import ExitStack

import concourse.bacc as bacc
import concourse.tile as tile
import concourse.mybir as mybir
from concourse.bass_utils import run_bass_kernel_spmd
from concourse.masks import make_identity

F32 = mybir.dt.float32
F32R = mybir.dt.float32r
EXP = mybir.ActivationFunctionType.Exp
TANH = mybir.ActivationFunctionType.Tanh
MUL = mybir.AluOpType.mult
ADD = mybir.AluOpType.add

BATCH, T, FEAT, HID = 64, 8192, 512, 256
NCORES = 8
SPC = BATCH // NCORES          # samples per core
NT = T // 128                  # 64 t-tiles per sample
GRP = 8                        # t-tiles per group
NG = NT // GRP                 # 8 groups per sample


def build_nc(reps: int = 1, dma_only: bool = False):
    nc = bacc.Bacc("TRN2", target_bir_lowering=False, debug=False)

    A = nc.dram_tensor("A", [SPC, FEAT], F32, kind="ExternalInput").ap()
    B = nc.dram_tensor("B", [SPC, T, HID], F32R, kind="ExternalInput").ap()
    W = nc.dram_tensor("W", [HID, FEAT], F32, kind="ExternalInput").ap()
    BIAS = nc.dram_tensor("BIAS", [HID], F32, kind="ExternalInput").ap()
    OUT = nc.dram_tensor("OUT", [SPC, HID], F32, kind="ExternalOutput").ap()

    Bv = B.rearrange("s (g j p) h -> s g p j h", g=NG, j=GRP, p=128)
    Wv = W.rearrange("(c p) f -> c p f", p=128)
    BIASv = BIAS.rearrange("(a h) -> a h", a=1)

    with tile.TileContext(nc) as tc, ExitStack() as ctx:
        cst = ctx.enter_context(tc.tile_pool(name="cst", bufs=1))
        ps_scr = ctx.enter_context(tc.tile_pool(name="ps_scr", bufs=1, space="PSUM"))

        ident = cst.tile([128, 128], F32, tag="ident")
        make_identity(nc, ident[:])
        zbias = cst.tile([128, 1], F32, tag="zbias")
        nc.gpsimd.memset(zbias[:], 0.0)
        ones_c = cst.tile([128, 1], F32, tag="ones_c")
        nc.gpsimd.memset(ones_c[:], 1.0)
        ones_row = cst.tile([1, 128], F32, tag="ones_row")
        nc.gpsimd.memset(ones_row[:], 1.0)
        act_scr = cst.tile([1, 1], F32, tag="act_scr")
        pool_scr = cst.tile([1, 1], F32, tag="pool_scr")
        dve_mark = cst.tile([1, 1], F32, tag="dve_mark")
        nc.gpsimd.memset(dve_mark[:], 0.0)
        # sel_all[k, s*128 + m] = 1 if k == s else 0 (per-sample broadcast selector)
        sel_all = cst.tile([SPC, SPC * 128], F32, tag="sel_all")
        nc.gpsimd.memset(sel_all[:], 0.0)
        nc.gpsimd.affine_select(
            out=sel_all[:].rearrange("k (s m) -> k s m", s=SPC),
            in_=sel_all[:].rearrange("k (s m) -> k s m", s=SPC),
            compare_op=mybir.AluOpType.not_equal,
            fill=1.0,
            base=0,
            pattern=[[-1, SPC], [0, 128]],
            channel_multiplier=1,
        )

        w_sb = cst.tile([128, 2 * FEAT], F32, tag="w_sb")
        nc.gpsimd.dma_start(w_sb[:, 0:FEAT], Wv[0])
        nc.gpsimd.dma_start(w_sb[:, FEAT : 2 * FEAT], Wv[1])
        a_sb = cst.tile([SPC, FEAT], F32, tag="a_sb")
        nc.gpsimd.dma_start(a_sb[:], A)
        b_row = cst.tile([1, HID], F32, tag="b_row")
        nc.gpsimd.dma_start(b_row[:], BIASv)

        scr = ps_scr.tile([1, 1], F32, tag="scr")

        def absorb(ap):
            nc.tensor.transpose(scr[:], ap, ident[0:1, 0:1])

        # ACT absorber for gpsimd-made constants (also loads the exp table set)
        nc.scalar.activation(act_scr[:], zbias[0:1, 0:1], EXP, bias=zbias[0:1, 0:1])
        # PE absorbers: gpsimd consts + the input DMA queues
        absorb(ident[0:1, 0:1])
        absorb(w_sb[0:1, 0:1])
        absorb(w_sb[0:1, FEAT : FEAT + 1])
        absorb(a_sb[0:1, 0:1])
        absorb(b_row[0:1, 0:1])

        # ---- AP = tanh(A @ W.T + bias) ----
        with tc.tile_pool(name="setup_ps", bufs=1, space="PSUM") as sps, \
             tc.tile_pool(name="setup_sb", bufs=1) as ssb:
            wt_sb = ssb.tile([128, 4 * HID], F32, tag="wt_sb")
            for fc in range(4):
                wt_ps = sps.tile([128, HID], F32, tag="wt_ps", name=f"wt_ps{fc}", bufs=2)
                for c in range(2):
                    nc.tensor.transpose(
                        wt_ps[:, c * 128 : (c + 1) * 128],
                        w_sb[:, c * FEAT + fc * 128 : c * FEAT + (fc + 1) * 128],
                        ident[:],
                    )
                nc.vector.tensor_copy(wt_sb[:, fc * HID : (fc + 1) * HID], wt_ps[:])

            at_ps = sps.tile([128, 4 * SPC], F32, tag="at_ps")
            for fc in range(4):
                nc.tensor.transpose(
                    at_ps[:, fc * SPC : (fc + 1) * SPC],
                    a_sb[0:SPC, fc * 128 : (fc + 1) * 128],
                    ident[0:SPC, 0:SPC],
                )
            at_sb = ssb.tile([128, 4 * SPC], F32, tag="at_sb")
            nc.vector.tensor_copy(at_sb[:], at_ps[:])

            ap_ps = sps.tile([SPC, HID], F32, tag="ap_ps")
            for fc in range(4):
                nc.tensor.matmul(
                    ap_ps[:],
                    at_sb[:, fc * SPC : (fc + 1) * SPC],
                    wt_sb[:, fc * HID : (fc + 1) * HID],
                    start=(fc == 0),
                    stop=False,
                )
            nc.tensor.matmul(ap_ps[:], ones_row[:, 0:SPC], b_row[:], start=False, stop=True)

            ap_sb = cst.tile([SPC, HID], F32, tag="ap_sb")
            nc.scalar.activation(ap_sb[:], ap_ps[:], TANH, bias=zbias[0:SPC])

            # broadcast every AP row to 128 partitions now: sel_s.T @ ap_sb
            apb_all = cst.tile([128, SPC * HID], F32, tag="apb_all")
            for _s in range(SPC):
                apb_ps = sps.tile([128, HID], F32, tag="apb_ps", name=f"apb_ps{_s}", bufs=2)
                nc.tensor.matmul(
                    apb_ps[:],
                    sel_all[0:SPC, _s * 128 : (_s + 1) * 128],
                    ap_sb[:],
                )
                nc.vector.tensor_copy(apb_all[:, _s * HID : (_s + 1) * HID], apb_ps[:])

            # AP transposed to [h, s] for the PE score path
            apt_ps = sps.tile([128, 2 * SPC], F32, tag="apt_ps")
            for hc in range(2):
                nc.tensor.transpose(
                    apt_ps[:, hc * SPC : (hc + 1) * SPC],
                    ap_sb[0:SPC, hc * 128 : (hc + 1) * 128],
                    ident[0:SPC, 0:SPC],
                )
            apt_sb = cst.tile([128, 2 * SPC], F32, tag="apt_sb")
            nc.vector.tensor_copy(apt_sb[:], apt_ps[:])

        # ---- main pools ----
        bpool = ctx.enter_context(tc.tile_pool(name="bpool", bufs=8))
        rpool = ctx.enter_context(tc.tile_pool(name="rpool", bufs=2))
        erpool = ctx.enter_context(tc.tile_pool(name="erpool", bufs=NG))
        small = ctx.enter_context(tc.tile_pool(name="small", bufs=2))
        con_pool = ctx.enter_context(tc.tile_pool(name="con", bufs=1))
        junk_pool = ctx.enter_context(tc.tile_pool(name="junk", bufs=1))
        g_pool = ctx.enter_context(tc.tile_pool(name="g_ps", bufs=2, space="PSUM"))
        tp_pool = ctx.enter_context(tc.tile_pool(name="tp_ps", bufs=2, space="PSUM"))
        rrow_pool = ctx.enter_context(tc.tile_pool(name="rrow_ps", bufs=2, space="PSUM"))
        eps_pool = ctx.enter_context(tc.tile_pool(name="e_ps", bufs=1, space="PSUM"))
        btg_pool = ctx.enter_context(tc.tile_pool(name="btg", bufs=2))
        erow_pool = ctx.enter_context(tc.tile_pool(name="erow", bufs=2))

        con_wide = con_pool.tile([1, SPC * HID], F32, tag="con_wide")
        junk = junk_pool.tile([128, HID], F32, tag="junk")

        started = False
        pending_tail = None
        for rep in range(reps):
            for s in range(SPC):
                if started and not dma_only:
                    prev = (s + SPC - 1) % SPC
                    absorb(con_wide[0:1, prev * HID : prev * HID + 1])

                ap_bcast = apb_all[:, s * HID : (s + 1) * HID]

                dacc = small.tile([128, NG], F32, tag="dacc")
                dacc2 = small.tile([1, NG], F32, tag="dacc2")
                G = g_pool.tile([1, HID + 8], F32, tag="G")

                prev_grp = None  # (e_r, bgrp, g) of previous group
                for g in range(NG):
                    if g == 1 and pending_tail is not None:
                        pending_tail()
                        pending_tail = None
                    if (started or g > 0) and not dma_only:
                        # absorb DVE tick on Pool so the B load carries only
                        # the PE (pooling WAR) wait
                        nc.gpsimd.tensor_copy(pool_scr[:], junk[0:1, 0:1])
                    bgrp = bpool.tile([128, GRP * HID], F32R, tag="bgrp")
                    nc.gpsimd.dma_start(
                        bgrp[:].rearrange("p (j h) -> p j h", j=GRP), Bv[s, g]
                    )

                    if dma_only:
                        prev_grp = None
                        continue
                    e_r = erpool.tile([128, GRP], F32R, tag="e_r")

                    # --- PE score path for tiles j=0,1 ---
                    # transposes laid out c-major so each h-chunk is contiguous
                    tp_ps = tp_pool.tile([128, 512], F32, tag="tp_ps")
                    for c in range(2):
                        for j in range(2):
                            nc.tensor.transpose(
                                tp_ps[:, (c * 2 + j) * 128 : (c * 2 + j + 1) * 128],
                                bgrp[:, j * HID + c * 128 : j * HID + (c + 1) * 128].bitcast(F32),
                                ident[:],
                            )
                    btg = btg_pool.tile([128, 512], F32, tag="btg")
                    nc.scalar.copy(btg[:], tp_ps[:])
                    r_row = rrow_pool.tile([1, 256], F32, tag="r_row")
                    for c in range(2):
                        nc.tensor.matmul(
                            r_row[:],
                            apt_sb[:, c * SPC + s : c * SPC + s + 1],
                            btg[:, c * 256 : (c + 1) * 256],
                            start=(c == 0),
                            stop=(c == 1),
                            skip_group_check=True,
                        )
                    e_row = erow_pool.tile([1, 256], F32, tag="e_row")
                    nc.scalar.activation(
                        e_row[:], r_row[:], EXP, scale=-1.0, bias=zbias[0:1],
                        accum_out=dacc2[0:1, g : g + 1],
                    )
                    e_ps = eps_pool.tile([128, 2], F32, tag="e_ps")
                    for j in range(2):
                        nc.tensor.transpose(
                            e_ps[:, j : j + 1],
                            e_row[0:1, j * 128 : (j + 1) * 128],
                            ident[0:1, 0:1],
                        )
                    for j in range(2):
                        nc.scalar.copy(e_r[:, j : j + 1], e_ps[:, j : j + 1])

                    # --- DVE score path for tiles j=2..7 ---
                    r_sb = rpool.tile([128, GRP], F32, tag="r_sb")
                    for j in range(2, GRP):
                        if j == 4 and prev_grp is not None:
                            per_, pbgrp, pg = prev_grp
                            for jj in range(GRP):
                                nc.tensor.matmul(
                                    G[0:1, 0:HID],
                                    per_[:, jj : jj + 1],
                                    pbgrp[:, jj * HID : (jj + 1) * HID],
                                    start=(pg == 0 and jj == 0),
                                    stop=False,
                                    skip_group_check=True,
                                )
                        nc.vector.scalar_tensor_tensor(
                            out=junk[:],
                            in0=bgrp[:, j * HID : (j + 1) * HID].bitcast(F32),
                            scalar=1.0,
                            in1=ap_bcast[:],
                            op0=MUL,
                            op1=MUL,
                            accum_out=r_sb[:, j : j + 1],
                        )

                    nc.scalar.activation(
                        e_r[:, 2:GRP], r_sb[:, 2:GRP], EXP, scale=-1.0, bias=zbias[:],
                        accum_out=dacc[:, g : g + 1],
                    )
                    prev_grp = (e_r, bgrp, g)
                    started = True

                # flush last group's pooling
                if dma_only:
                    continue
                per_, pbgrp, pg = prev_grp
                for jj in range(GRP):
                    nc.tensor.matmul(
                        G[0:1, 0:HID],
                        per_[:, jj : jj + 1],
                        pbgrp[:, jj * HID : (jj + 1) * HID],
                        start=False,
                        stop=False,
                        skip_group_check=True,
                    )

                # den/scale tail: defer into the next sample's stream
                def make_tail(dacc=dacc, dacc2=dacc2, G=G, s=s):
                    def tail():
                        dsum = small.tile([128, 1], F32, tag="dsum", name="dsum")
                        nc.vector.tensor_reduce(
                            dsum[:], dacc[:], axis=mybir.AxisListType.X, op=ADD
                        )
                        nc.tensor.matmul(
                            G[0:1, HID : HID + 1], ones_c[:], dsum[:],
                            start=False, stop=False, skip_group_check=True,
                        )
                        d2s = small.tile([1, 1], F32, tag="d2s", name="d2s")
                        nc.vector.tensor_reduce(
                            d2s[:], dacc2[:], axis=mybir.AxisListType.X, op=ADD
                        )
                        nc.tensor.matmul(
                            G[0:1, HID : HID + 1], ones_c[0:1, :], d2s[:],
                            start=False, stop=True, skip_group_check=True,
                        )
                        inv = small.tile([1, 1], F32, tag="inv", name="inv")
                        nc.vector.reciprocal(inv[:], G[0:1, HID : HID + 1])
                        nc.vector.tensor_scalar_mul(
                            con_wide[0:1, s * HID : (s + 1) * HID], G[0:1, 0:HID], inv[:]
                        )
                        nc.vector.tensor_copy(dve_mark[:], inv[:])
                    return tail
                pending_tail = make_tail()

        if pending_tail is not None:
            pending_tail()
            pending_tail = None
        if dma_only:
            nc.vector.tensor_copy(con_wide[0:1, 0:1], zbias[0:1, 0:1])
        OUTv = OUT.rearrange("s h -> (s h)").rearrange("(a n) -> a n", a=1)
        nc.gpsimd.dma_start(OUTv, con_wide[0:1, :])

    nc.compile()
    return nc


_NC_CACHE = {}


def _get_nc(reps: int = 1):
    if reps not in _NC_CACHE:
        _NC_CACHE[reps] = build_nc(reps)
    return _NC_CACHE[reps]


def kernel(A, B, W, b):
    A = np.asarray(A, dtype=np.float32)
    B = np.asarray(B, dtype=np.float32)
    W = np.asarray(W, dtype=np.float32)
    b = np.asarray(b, dtype=np.float32)

    nc = _get_nc(1)
    in_maps = [
        {
            "A": A[c * SPC : (c + 1) * SPC],
            "B": B[c * SPC : (c + 1) * SPC],
            "W": W,
            "BIAS": b,
        }
        for c in range(NCORES)
    ]
    res = run_bass_kernel_spmd(nc, in_maps, list(range(NCORES))).results
    out = np.concatenate([r["OUT"] for r in res], axis=0)
    return out[:, None, :].astype(np.float32)
